# revision 12
# baseline (speedup 1.0000x reference)
"""Causal multi-head attention block (B=16, S=1024, d=1024, H=16) on 8 NeuronCores.

Strategy: data-parallel over batch (2 batches per core), no collectives.
Per-core kernel (fp16 matmuls, fp32 accumulation), fully software-pipelined
so the PE array never starves (keeps the 2.4GHz p-state):

  - All transposes are fp16 (DVE pre-casts the fp32 DMA tiles), 4x cheaper
    on PE than fp32 transposes at the degraded p-state.
  - Emission is generator-based: projection/transpose work for batch 1 is
    "filler" pumped between score and AV matmuls of batch-0 attention (the
    ACT-bound stretches), and batch-0 output-projection fills batch-1
    attention.  The PE instruction stream always has ready work.
  - Attention per (batch, head, q-chunk): scoresT[k,q] = KT.T @ QT on PE
    into 2-bank PSUM pair tiles, exp(s/8) on ACT per k-tile, causal mask
    via 0/1 triangle multiply on diagonal blocks, then out_unT[dh,q]
    (+ sum row, via a fused ones column) = [V|1].T @ expT accumulated on
    PE.  Per 4 heads one batched DVE reciprocal computes 1/sums; chunked
    f16 DMA broadcasts fan them out and one in-place DVE multiply per
    (pair, qc) normalizes (ACT stays exp-only).
  - The last head group of batch 1 runs qc-major with split normalization
    so the final output-projection tiles start earlier.
  - y is produced in f16 and upcast on the host.
Biases: bq/bk are zero by problem spec (ignored); bv/bo folded in exactly
on the host (y += bv @ Wo.T + bo).
"""

import numpy as np

_CACHE: dict = {}

S = 1024
D = 1024
H = 16
DH = 64
BPC = 2           # batches per core
M = BPC * S       # tokens per core
NCORES = 8
NDT = D // 128    # 8 d-tiles

_SENT = object()


def _build_nc():
    import concourse.bass as bass  # noqa: F401
    import concourse.mybir as mybir
    import concourse.tile as tile
    from concourse import bacc
    from concourse.masks import make_identity
    from contextlib import ExitStack
    from itertools import chain

    f32 = mybir.dt.float32
    f16 = mybir.dt.float16
    EXPF = mybir.ActivationFunctionType.Exp

    nc = bacc.Bacc("TRN2", target_bir_lowering=False, debug=False,
                   num_devices=NCORES)

    x_d = nc.dram_tensor("x", [M, D], f32, kind="ExternalInput")
    wq_d = nc.dram_tensor("Wq", [D, D], f32, kind="ExternalInput")
    wk_d = nc.dram_tensor("Wk", [D, D], f32, kind="ExternalInput")
    wv_d = nc.dram_tensor("Wv", [D, D], f32, kind="ExternalInput")
    wo_d = nc.dram_tensor("Wo", [D, D], f32, kind="ExternalInput")
    tri_d = nc.dram_tensor("tri01", [128, 128], f16, kind="ExternalInput")
    y_d = nc.dram_tensor("y", [M, D], f16, kind="ExternalOutput")

    with tile.TileContext(nc) as tc, ExitStack() as top:
        consts = top.enter_context(tc.tile_pool(name="consts", bufs=1))
        persist = top.enter_context(tc.tile_pool(name="persist", bufs=1))
        xtao = top.enter_context(tc.tile_pool(name="xtao", bufs=2))
        wt = top.enter_context(tc.tile_pool(name="wt", bufs=3))
        stage = top.enter_context(tc.tile_pool(name="stage", bufs=3))
        xcp = top.enter_context(tc.tile_pool(name="xcp", bufs=3))
        ystage = top.enter_context(tc.tile_pool(name="ystage", bufs=2))
        expp = top.enter_context(tc.tile_pool(name="expp", bufs=2))
        srp = top.enter_context(tc.tile_pool(name="srp", bufs=1))
        bcp = top.enter_context(tc.tile_pool(name="bcp", bufs=1))
        grpp = top.enter_context(tc.tile_pool(name="grpp", bufs=2))
        nrmp = top.enter_context(tc.tile_pool(name="nrmp", bufs=1))
        tmpp = top.enter_context(tc.tile_pool(name="tmpp", bufs=1))
        # PSUM: 2 + 2*2 + 2 = 8 banks
        psAcc = top.enter_context(tc.tile_pool(name="psAcc", bufs=2, space="PSUM"))
        psS = top.enter_context(tc.tile_pool(name="psS", bufs=2, space="PSUM"))
        psO = top.enter_context(tc.tile_pool(name="psO", bufs=2, space="PSUM"))

        ident = consts.tile([128, 128], f16, tag="ident")
        make_identity(nc, ident)
        tri01 = consts.tile([128, 128], f16, tag="tri")
        nc.sync.dma_start(out=tri01, in_=tri_d[:, :])

        # persistent activations (fp16)
        QT = persist.tile([128, NDT, M], f16, tag="QT")    # [o, m] transposed
        KT = persist.tile([128, NDT, M], f16, tag="KT")
        V = persist.tile([128, M // 128, H * 65], f16, tag="V")

        # x transposed, per batch; AO tiles reuse these slots once the
        # corresponding projections are done (ring of 2)
        xT0 = xtao.tile([128, NDT, S], f16, tag="xTAO")
        xT1 = xtao.tile([128, NDT, S], f16, tag="xTAO")

        evac_rr = [0]

        def evac_copy(out, in_, eng=None):
            """PSUM->SBUF evacuation; eng: 'a' ACT, 'v' DVE, None alternates."""
            if eng is None:
                eng = "a" if evac_rr[0] % 3 == 0 else "v"
                evac_rr[0] += 1
            if eng == "a":
                nc.scalar.copy(out=out, in_=in_)
            else:
                nc.vector.tensor_copy(out=out, in_=in_)

        def g_load_transposed(dst, dram, ncols, row0=0, eng=None):
            """generator: dst[:, ib, rt*128:(rt+1)*128] = dram[row0+rt*128:
            +128, :].T   (fp16; dram fp32).  One yield per 4-transpose
            half-row (one DMA + DVE cast + 4 PE transposes + 1 evac)."""
            for rt in range(ncols // 128):
                for g in range(2):
                    st = stage.tile([128, 512], f32, tag="stage")
                    nc.sync.dma_start(
                        out=st,
                        in_=dram[row0 + rt * 128:row0 + (rt + 1) * 128,
                                 g * 512:(g + 1) * 512])
                    xc = xcp.tile([128, 512], f16, tag="xc")
                    nc.vector.tensor_copy(out=xc, in_=st)
                    pt = psAcc.tile([128, 512], f16, tag="psP")
                    for c in range(4):
                        nc.tensor.transpose(
                            pt[:, c * 128:(c + 1) * 128],
                            xc[:, c * 128:(c + 1) * 128], ident)
                    evac_copy(dst[:, g * 4:g * 4 + 4, rt * 128:(rt + 1) * 128],
                              pt.rearrange("p (a b) -> p a b", b=128), eng)
                    yield

        def g_proj_qk(WT, xTb, dst, b, eng=None):
            """generator: dst[:, ot, b*S + ...] = W @ x_b.T (transposed
            layout).  Yields every 2 accumulating matmuls."""
            for mc in range(2):
                c0 = b * S + mc * 512
                for ot in range(NDT):
                    pp = psAcc.tile([128, 512], f32, tag="psP")
                    for it in range(NDT):
                        nc.tensor.matmul(
                            pp,
                            WT[:, it, ot * 128:(ot + 1) * 128],
                            xTb[:, it, mc * 512:(mc + 1) * 512],
                            start=(it == 0), stop=(it == NDT - 1))
                        if it % 2 == 1:
                            yield
                    evac_copy(dst[:, ot, c0:c0 + 512], pp, eng)

        def g_proj_v(WvT, xTb, b, eng=None):
            """generator: V[:, b*8+mt, strips] = x_b @ Wv.T (natural layout,
            65-wide head strips with a fused ones column)."""
            for mt in range(8):
                mtv = b * 8 + mt
                v2 = V[:, mtv, :].rearrange("p (a c) -> p a c", c=65)
                nc.gpsimd.memset(v2[:, :, 64], 1.0)
                for oc in range(2):
                    pp = psAcc.tile([128, 512], f32, tag="psP")
                    for it in range(NDT):
                        nc.tensor.matmul(
                            pp,
                            xTb[:, it, mt * 128:(mt + 1) * 128],
                            WvT[:, it, oc * 512:(oc + 1) * 512],
                            start=(it == 0), stop=(it == NDT - 1))
                        if it % 2 == 1:
                            yield
                    evac_copy(v2[:, 8 * oc:8 * oc + 8, 0:64],
                              pp.rearrange("p (a c) -> p a c", c=64), eng)

        def g_outproj(AOb, WoT, b, mts, eng=None):
            """generator: y[b*S + mt*128 ...] = AO_b.T @ WoT -> DRAM (f16)."""
            for mt in mts:
                ys = ystage.tile([128, D], f16, tag="ys")
                for oc in range(2):
                    pp = psAcc.tile([128, 512], f32, tag="psP")
                    for dt_ in range(NDT):
                        nc.tensor.matmul(
                            pp,
                            AOb[:, dt_, mt * 128:(mt + 1) * 128],
                            WoT[:, dt_, oc * 512:(oc + 1) * 512],
                            start=(dt_ == 0), stop=(dt_ == NDT - 1))
                        if dt_ % 2 == 1:
                            yield
                    evac_copy(ys[:, oc * 512:(oc + 1) * 512], pp, eng)
                nc.sync.dma_start(
                    out=y_d[b * S + mt * 128:b * S + (mt + 1) * 128, :],
                    in_=ys)
                yield

        def drain(*gens):
            """round-robin-drain generators (zip-interleaved emission)."""
            live = list(gens)
            while live:
                for g in list(live):
                    if next(g, _SENT) is _SENT:
                        live.remove(g)

        def drain2(main, aux, ratio=2):
            """drain main and aux interleaved, ratio main units per aux
            unit (main is PE-heavy projection work, aux is DMA-paced)."""
            while True:
                done_m = False
                for _ in range(ratio):
                    if next(main, _SENT) is _SENT:
                        done_m = True
                        break
                if next(aux, _SENT) is _SENT:
                    if done_m:
                        break
                    drain(main)
                    break
                if done_m:
                    drain(aux)
                    break

        # ---------------- attention ----------------
        # sum-row layout in sgrp: row = qc*4 + (h%4)

        def _bcast_mul(AOb, rg, hgrp, qc):
            """fan rg rows out to 128 partitions (DMA) and normalize the
            two AO head-pair blocks of (group, qc) in place (DVE)."""
            q0 = qc * 512
            for lp in range(2):
                p = 2 * hgrp + lp
                loc_e = qc * 4 + 2 * lp
                loc_o = loc_e + 1
                bc = bcp.tile([128, 512], f16, tag="bc")
                for loc, p0 in ((loc_e, 0), (loc_o, 64)):
                    r1 = rg[loc:loc + 1, :]
                    for ch in range(4):
                        rc = r1[:, ch * 128:(ch + 1) * 128]
                        rsrc = bass.AP(
                            tensor=rc.tensor, offset=rc.offset,
                            ap=[list(rc.ap[0]), [0, 64]]
                            + [list(a) for a in rc.ap[1:]])
                        nc.sync.dma_start(
                            out=bc[p0:p0 + 64, ch * 128:(ch + 1) * 128],
                            in_=rsrc)
                nc.vector.tensor_mul(
                    out=AOb[:, p, q0:q0 + 512],
                    in0=AOb[:, p, q0:q0 + 512], in1=bc)

        def normalize_group(AOb, hgrp, sgrp):
            """Reciprocal + normalize both qcs of a 4-head group."""
            rg32 = nrmp.tile([8, 512], f32, tag="rg32")
            rg = nrmp.tile([8, 512], f16, tag="rg")
            nc.vector.reciprocal_approx_fast(out=rg32, in_=sgrp)
            nc.vector.tensor_copy(out=rg, in_=rg32)
            for qc in range(2):
                _bcast_mul(AOb, rg, hgrp, qc)

        def normalize_qc(AOb, hgrp, sgrp, qc):
            """split variant: normalize only one qc of a 4-head group."""
            rg32 = nrmp.tile([8, 512], f32, tag="rg32")
            rg = nrmp.tile([8, 512], f16, tag="rg")
            # partition slices must be 32-aligned, so process all 8 rows;
            # the other qc's rows are garbage here but never read
            nc.vector.reciprocal_approx_fast(out=rg32, in_=sgrp)
            nc.vector.tensor_copy(out=rg, in_=rg32)
            _bcast_mul(AOb, rg, hgrp, qc)

        def attn_hqc(b, h, qc, AOb, sgrp, pump):
            """scores + exp + mask + AV + evac for one (batch, head, qc).
            pump(n) emits ~n x 0.4us of filler PE work."""
            thq = h // 2
            po = (h % 2) * 64
            even = (h % 2 == 0)
            loc = qc * 4 + (h % 4)
            q0 = b * S + qc * 512          # global m coords for QT/KT
            ql = qc * 512                  # batch-local q for AO
            nkt = (qc + 1) * 4
            ps_o = psO.tile([128, 512], f32, tag="psO")
            exts = [None] * nkt
            av_done = [0]

            def emit_avs(upto):
                while av_done[0] < upto:
                    kt = av_done[0]
                    ex, c0, off = exts[kt]
                    nc.tensor.matmul(
                        ps_o[0:65, off:512],
                        V[:, b * 8 + kt, h * 65:h * 65 + 65],
                        ex[:, c0:c0 + 512 - off],
                        start=(kt == 0), stop=(kt == nkt - 1))
                    av_done[0] += 1

            for pr in range(nkt // 2):
                ps_s = psS.tile([128, 1024], f32, tag="psS")
                for j in (0, 1):
                    kt = 2 * pr + j
                    k0 = kt * 128
                    off = max(0, k0 - qc * 512)
                    nc.tensor.matmul(
                        ps_s[:, j * 512 + off:j * 512 + 512],
                        KT[po:po + 64, thq, b * S + k0:b * S + k0 + 128],
                        QT[po:po + 64, thq, q0 + off:q0 + 512],
                        start=True, stop=True)
                ex = expp.tile([128, 1024], f16, tag="exp")
                offs = [max(0, (2 * pr + j) * 128 - qc * 512) for j in (0, 1)]
                if offs[0] == 0:
                    # gap-free pair: one wide exp (cols 512..512+off_b are
                    # stale psum; exp of them lands in unread ex cols)
                    nc.scalar.activation(
                        out=ex[:, 0:1024], in_=ps_s[:, 0:1024],
                        func=EXPF, scale=0.125)
                else:
                    for j in (0, 1):
                        nc.scalar.activation(
                            out=ex[:, j * 512 + offs[j]:(j + 1) * 512],
                            in_=ps_s[:, j * 512 + offs[j]:(j + 1) * 512],
                            func=EXPF, scale=0.125)
                for j in (0, 1):
                    kt = 2 * pr + j
                    off = offs[j]
                    if kt * 128 >= qc * 512:  # diagonal block: 0/1 triangle
                        nc.gpsimd.tensor_mul(
                            ex[:, j * 512 + off:j * 512 + off + 128],
                            ex[:, j * 512 + off:j * 512 + off + 128], tri01)
                    exts[kt] = (ex, j * 512 + off, off)
                if pr == 0:
                    pump(2)
                else:
                    emit_avs(2 * pr)
                    pump(1)
            emit_avs(nkt)

            # evacuate unnormalized output + sum row
            srow = srp.tile([65, 512], f32, tag="srow")
            nc.vector.tensor_copy(out=srow[64:65, :], in_=ps_o[64:65, :])
            nc.sync.dma_start(out=sgrp[loc:loc + 1, :], in_=srow[64:65, :])
            if even:
                nc.vector.tensor_copy(
                    out=AOb[0:64, thq, ql:ql + 512], in_=ps_o[0:64, :])
            else:
                tmp = tmpp.tile([64, 512], f16, tag="tmp")
                nc.vector.tensor_copy(out=tmp, in_=ps_o[0:64, :])
                nc.sync.dma_start(
                    out=AOb[64:128, thq, ql:ql + 512], in_=tmp)
            pump(1)

        # ---------------- emission schedule ----------------
        # phases A/B for batch 0, zip-interleaved so PE has work while the
        # input DMAs stream
        WvT = wt.tile([128, NDT, D], f16, tag="WT")
        drain(g_load_transposed(WvT, wv_d, D),
              g_load_transposed(xT0, x_d, S, row0=0))
        WqT = wt.tile([128, NDT, D], f16, tag="WT")
        drain2(g_proj_v(WvT, xT0, 0), g_load_transposed(WqT, wq_d, D))
        WkT = wt.tile([128, NDT, D], f16, tag="WT")
        drain2(g_proj_qk(WqT, xT0, QT, 0), g_load_transposed(WkT, wk_d, D))
        drain2(g_proj_qk(WkT, xT0, KT, 0),
               g_load_transposed(xT1, x_d, S, row0=S))

        # filler stream pumped between attention matmuls
        WoT = wt.tile([128, NDT, D], f16, tag="WT")  # ring slot of WqT
        fill = [chain(
            g_proj_qk(WqT, xT1, QT, 1),
            g_proj_qk(WkT, xT1, KT, 1),
            g_proj_v(WvT, xT1, 1),
            g_load_transposed(WoT, wo_d, D),
        )]

        def pump(n):
            for _ in range(n):
                if next(fill[0], _SENT) is _SENT:
                    return

        # attention batch 0 (QKV-b1 + WoT prep as filler)
        AO0 = xtao.tile([128, NDT, S], f16, tag="xTAO")  # ring slot of xT0
        for h in range(H):
            if h % 4 == 0:
                sgrp = grpp.tile([8, 512], f32, tag="sgrp")
            for qc in range(2):
                attn_hqc(0, h, qc, AO0, sgrp, pump)
            if h % 4 == 3:
                normalize_group(AO0, h // 4, sgrp)

        # drain remaining batch-1 projection work: it must complete before
        # AO1 (ring slot of xT1) can be written, or the psO ring deadlocks
        drain(fill[0])

        # attention batch 1 (out-proj b0 as filler); last head group runs
        # qc-major with split normalization so out-proj b1 starts early
        AO1 = xtao.tile([128, NDT, S], f16, tag="xTAO")  # ring slot of xT1
        fill[0] = g_outproj(AO0, WoT, 0, range(8))
        for h in range(12):
            if h % 4 == 0:
                sgrp = grpp.tile([8, 512], f32, tag="sgrp")
            for qc in range(2):
                attn_hqc(1, h, qc, AO1, sgrp, pump)
            if h % 4 == 3:
                normalize_group(AO1, h // 4, sgrp)
        sgrp = grpp.tile([8, 512], f32, tag="sgrp")
        for qc in range(2):
            for h in range(12, 16):
                attn_hqc(1, h, qc, AO1, sgrp, pump)
                if qc == 1 and h == 13:
                    # norm of qc0 is complete by now; out-proj b1 for the
                    # first m-half can fill the rest of the qc1 pass
                    fill[0] = chain(fill[0],
                                    g_outproj(AO1, WoT, 1, range(4)))
            normalize_qc(AO1, 3, sgrp, qc)

        # drain leftovers + final out-proj tiles
        drain(fill[0], g_outproj(AO1, WoT, 1, range(4, 8)))

    nc.compile()
    return nc


def _tri01():
    # tri01[dk, dq] = 1 where k <= q (allowed), else 0
    return np.triu(np.ones((128, 128), np.float16))


def _get_nc():
    if "nc" not in _CACHE:
        _CACHE["nc"] = _build_nc()
    return _CACHE["nc"]


def kernel(x, Wq, bq, Wk, bk, Wv, bv, Wo, bo):
    from concourse.bass_utils import run_bass_kernel_spmd

    x = np.ascontiguousarray(np.asarray(x, dtype=np.float32))
    B = x.shape[0]
    assert x.shape == (B, S, D) and B == NCORES * BPC
    Wq = np.ascontiguousarray(np.asarray(Wq, dtype=np.float32))
    Wk = np.ascontiguousarray(np.asarray(Wk, dtype=np.float32))
    Wv = np.ascontiguousarray(np.asarray(Wv, dtype=np.float32))
    Wo = np.ascontiguousarray(np.asarray(Wo, dtype=np.float32))

    nc = _get_nc()
    shards = x.reshape(NCORES, M, D)
    tri = _tri01()
    in_maps = [
        {"x": shards[c], "Wq": Wq, "Wk": Wk, "Wv": Wv, "Wo": Wo, "tri01": tri}
        for c in range(NCORES)
    ]
    res = run_bass_kernel_spmd(nc, in_maps, core_ids=list(range(NCORES)))
    y = np.stack([res.results[c]["y"] for c in range(NCORES)])
    y = y.reshape(B, S, D).astype(np.float32)

    # exact host-side fold of bv and bo (bq/bk are zero by problem spec)
    bias = (np.asarray(bv, np.float32) @ np.asarray(Wo, np.float32).T
            + np.asarray(bo, np.float32))
    if np.any(bias):
        y = y + bias
    return y.astype(np.float32)


# revision 13
# speedup vs baseline: 1.0460x; 1.0460x over previous
"""Causal multi-head attention block (B=16, S=1024, d=1024, H=16) on 8 NeuronCores.

Strategy: data-parallel over batch (2 batches per core), no collectives.
Per-core kernel (fp16 matmuls, fp32 accumulation), fully software-pipelined
so the PE array never starves (keeps the 2.4GHz p-state):

  - All transposes are fp16 (DVE pre-casts the fp32 DMA tiles), 4x cheaper
    on PE than fp32 transposes at the degraded p-state.
  - Emission is generator-based: projection/transpose work for batch 1 is
    "filler" pumped between score and AV matmuls of batch-0 attention (the
    ACT-bound stretches), and batch-0 output-projection fills batch-1
    attention.  The PE instruction stream always has ready work.
  - Attention per (batch, head, q-chunk): scoresT[k,q] = KT.T @ QT on PE
    into 2-bank PSUM pair tiles, exp(s/8) on ACT per k-tile, causal mask
    via 0/1 triangle multiply on diagonal blocks, then out_unT[dh,q]
    (+ sum row, via a fused ones column) = [V|1].T @ expT accumulated on
    PE.  Per 4 heads one batched DVE reciprocal computes 1/sums; chunked
    f16 DMA broadcasts fan them out and one in-place DVE multiply per
    (pair, qc) normalizes (ACT stays exp-only).
  - The last head group of batch 1 runs qc-major with split normalization
    so the final output-projection tiles start earlier.
  - y is produced in f16 and upcast on the host.
Biases: bq/bk are zero by problem spec (ignored); bv/bo folded in exactly
on the host (y += bv @ Wo.T + bo).
"""

import numpy as np

_CACHE: dict = {}

S = 1024
D = 1024
H = 16
DH = 64
BPC = 2           # batches per core
M = BPC * S       # tokens per core
NCORES = 8
NDT = D // 128    # 8 d-tiles

_SENT = object()


def _build_nc():
    import concourse.bass as bass  # noqa: F401
    import concourse.mybir as mybir
    import concourse.tile as tile
    from concourse import bacc
    from concourse.masks import make_identity
    from contextlib import ExitStack
    from itertools import chain

    f32 = mybir.dt.float32
    f16 = mybir.dt.float16
    EXPF = mybir.ActivationFunctionType.Exp

    nc = bacc.Bacc("TRN2", target_bir_lowering=False, debug=False,
                   num_devices=NCORES)

    x_d = nc.dram_tensor("x", [M, D], f32, kind="ExternalInput")
    wq_d = nc.dram_tensor("Wq", [D, D], f32, kind="ExternalInput")
    wk_d = nc.dram_tensor("Wk", [D, D], f32, kind="ExternalInput")
    wv_d = nc.dram_tensor("Wv", [D, D], f32, kind="ExternalInput")
    wo_d = nc.dram_tensor("Wo", [D, D], f32, kind="ExternalInput")
    tri_d = nc.dram_tensor("tri01", [128, 128], f16, kind="ExternalInput")
    y_d = nc.dram_tensor("y", [M, D], f16, kind="ExternalOutput")

    with tile.TileContext(nc) as tc, ExitStack() as top:
        consts = top.enter_context(tc.tile_pool(name="consts", bufs=1))
        persist = top.enter_context(tc.tile_pool(name="persist", bufs=1))
        xtao = top.enter_context(tc.tile_pool(name="xtao", bufs=2))
        wt = top.enter_context(tc.tile_pool(name="wt", bufs=3))
        stage = top.enter_context(tc.tile_pool(name="stage", bufs=3))
        xcp = top.enter_context(tc.tile_pool(name="xcp", bufs=3))
        ystage = top.enter_context(tc.tile_pool(name="ystage", bufs=2))
        expp = top.enter_context(tc.tile_pool(name="expp", bufs=4))
        srp = top.enter_context(tc.tile_pool(name="srp", bufs=1))
        bcp = top.enter_context(tc.tile_pool(name="bcp", bufs=1))
        grpp = top.enter_context(tc.tile_pool(name="grpp", bufs=2))
        nrmp = top.enter_context(tc.tile_pool(name="nrmp", bufs=1))
        tmpp = top.enter_context(tc.tile_pool(name="tmpp", bufs=1))
        # PSUM: 2 + 2*2 + 2 = 8 banks
        psAcc = top.enter_context(tc.tile_pool(name="psAcc", bufs=2, space="PSUM"))
        psS = top.enter_context(tc.tile_pool(name="psS", bufs=2, space="PSUM"))
        psO = top.enter_context(tc.tile_pool(name="psO", bufs=2, space="PSUM"))

        ident = consts.tile([128, 128], f16, tag="ident")
        make_identity(nc, ident)
        tri01 = consts.tile([128, 128], f16, tag="tri")
        nc.sync.dma_start(out=tri01, in_=tri_d[:, :])

        # persistent activations (fp16)
        QT = persist.tile([128, NDT, M], f16, tag="QT")    # [o, m] transposed
        KT = persist.tile([128, NDT, M], f16, tag="KT")
        V = persist.tile([128, M // 128, H * 65], f16, tag="V")

        # x transposed, per batch; AO tiles reuse these slots once the
        # corresponding projections are done (ring of 2)
        xT0 = xtao.tile([128, NDT, S], f16, tag="xTAO")
        xT1 = xtao.tile([128, NDT, S], f16, tag="xTAO")

        evac_rr = [0]

        def evac_copy(out, in_, eng=None):
            """PSUM->SBUF evacuation; eng: 'a' ACT, 'v' DVE, None alternates."""
            if eng is None:
                eng = "a" if evac_rr[0] % 2 == 0 else "v"
                evac_rr[0] += 1
            if eng == "a":
                nc.scalar.copy(out=out, in_=in_)
            else:
                nc.vector.tensor_copy(out=out, in_=in_)

        def g_load_transposed(dst, dram, ncols, row0=0, eng=None):
            """generator: dst[:, ib, rt*128:(rt+1)*128] = dram[row0+rt*128:
            +128, :].T   (fp16; dram fp32).  One yield per 4-transpose
            half-row (one DMA + DVE cast + 4 PE transposes + 1 evac)."""
            for rt in range(ncols // 128):
                for g in range(2):
                    st = stage.tile([128, 512], f32, tag="stage")
                    nc.sync.dma_start(
                        out=st,
                        in_=dram[row0 + rt * 128:row0 + (rt + 1) * 128,
                                 g * 512:(g + 1) * 512])
                    xc = xcp.tile([128, 512], f16, tag="xc")
                    nc.vector.tensor_copy(out=xc, in_=st)
                    pt = psAcc.tile([128, 512], f16, tag="psP")
                    for c in range(4):
                        nc.tensor.transpose(
                            pt[:, c * 128:(c + 1) * 128],
                            xc[:, c * 128:(c + 1) * 128], ident)
                    evac_copy(dst[:, g * 4:g * 4 + 4, rt * 128:(rt + 1) * 128],
                              pt.rearrange("p (a b) -> p a b", b=128), eng)
                    yield

        def g_proj_qk(WT, xTb, dst, b, eng=None):
            """generator: dst[:, ot, b*S + ...] = W @ x_b.T (transposed
            layout).  Yields every 2 accumulating matmuls."""
            for mc in range(2):
                c0 = b * S + mc * 512
                for ot in range(NDT):
                    pp = psAcc.tile([128, 512], f32, tag="psP")
                    for it in range(NDT):
                        nc.tensor.matmul(
                            pp,
                            WT[:, it, ot * 128:(ot + 1) * 128],
                            xTb[:, it, mc * 512:(mc + 1) * 512],
                            start=(it == 0), stop=(it == NDT - 1))
                        if it % 2 == 1:
                            yield
                    evac_copy(dst[:, ot, c0:c0 + 512], pp, eng)

        def g_proj_v(WvT, xTb, b, eng=None):
            """generator: V[:, b*8+mt, strips] = x_b @ Wv.T (natural layout,
            65-wide head strips with a fused ones column)."""
            for mt in range(8):
                mtv = b * 8 + mt
                v2 = V[:, mtv, :].rearrange("p (a c) -> p a c", c=65)
                nc.gpsimd.memset(v2[:, :, 64], 1.0)
                for oc in range(2):
                    pp = psAcc.tile([128, 512], f32, tag="psP")
                    for it in range(NDT):
                        nc.tensor.matmul(
                            pp,
                            xTb[:, it, mt * 128:(mt + 1) * 128],
                            WvT[:, it, oc * 512:(oc + 1) * 512],
                            start=(it == 0), stop=(it == NDT - 1))
                        if it % 2 == 1:
                            yield
                    evac_copy(v2[:, 8 * oc:8 * oc + 8, 0:64],
                              pp.rearrange("p (a c) -> p a c", c=64), eng)

        def g_outproj(AOb, WoT, b, mts, eng=None):
            """generator: y[b*S + mt*128 ...] = AO_b.T @ WoT -> DRAM (f16)."""
            for mt in mts:
                ys = ystage.tile([128, D], f16, tag="ys")
                for oc in range(2):
                    pp = psAcc.tile([128, 512], f32, tag="psP")
                    for dt_ in range(NDT):
                        nc.tensor.matmul(
                            pp,
                            AOb[:, dt_, mt * 128:(mt + 1) * 128],
                            WoT[:, dt_, oc * 512:(oc + 1) * 512],
                            start=(dt_ == 0), stop=(dt_ == NDT - 1))
                        if dt_ % 2 == 1:
                            yield
                    evac_copy(ys[:, oc * 512:(oc + 1) * 512], pp, eng)
                nc.sync.dma_start(
                    out=y_d[b * S + mt * 128:b * S + (mt + 1) * 128, :],
                    in_=ys)
                yield

        def drain(*gens):
            """round-robin-drain generators (zip-interleaved emission)."""
            live = list(gens)
            while live:
                for g in list(live):
                    if next(g, _SENT) is _SENT:
                        live.remove(g)

        def drain2(main, aux, ratio=2):
            """drain main and aux interleaved, ratio main units per aux
            unit (main is PE-heavy projection work, aux is DMA-paced)."""
            while True:
                done_m = False
                for _ in range(ratio):
                    if next(main, _SENT) is _SENT:
                        done_m = True
                        break
                if next(aux, _SENT) is _SENT:
                    if done_m:
                        break
                    drain(main)
                    break
                if done_m:
                    drain(aux)
                    break

        # ---------------- attention ----------------
        # sum-row layout in sgrp: row = qc*4 + (h%4)

        def _bcast_mul(AOb, rg, hgrp, qc):
            """fan rg rows out to 128 partitions (DMA) and normalize the
            two AO head-pair blocks of (group, qc) in place (DVE)."""
            q0 = qc * 512
            for lp in range(2):
                p = 2 * hgrp + lp
                loc_e = qc * 4 + 2 * lp
                loc_o = loc_e + 1
                bc = bcp.tile([128, 512], f16, tag="bc")
                for loc, p0 in ((loc_e, 0), (loc_o, 64)):
                    r1 = rg[loc:loc + 1, :]
                    for ch in range(4):
                        rc = r1[:, ch * 128:(ch + 1) * 128]
                        rsrc = bass.AP(
                            tensor=rc.tensor, offset=rc.offset,
                            ap=[list(rc.ap[0]), [0, 64]]
                            + [list(a) for a in rc.ap[1:]])
                        nc.sync.dma_start(
                            out=bc[p0:p0 + 64, ch * 128:(ch + 1) * 128],
                            in_=rsrc)
                nc.vector.tensor_mul(
                    out=AOb[:, p, q0:q0 + 512],
                    in0=AOb[:, p, q0:q0 + 512], in1=bc)

        def normalize_group(AOb, hgrp, sgrp):
            """Reciprocal + normalize both qcs of a 4-head group."""
            rg32 = nrmp.tile([8, 512], f32, tag="rg32")
            rg = nrmp.tile([8, 512], f16, tag="rg")
            nc.vector.reciprocal_approx_fast(out=rg32, in_=sgrp)
            nc.vector.tensor_copy(out=rg, in_=rg32)
            for qc in range(2):
                _bcast_mul(AOb, rg, hgrp, qc)

        def normalize_qc(AOb, hgrp, sgrp, qc):
            """split variant: normalize only one qc of a 4-head group."""
            rg32 = nrmp.tile([8, 512], f32, tag="rg32")
            rg = nrmp.tile([8, 512], f16, tag="rg")
            # partition slices must be 32-aligned, so process all 8 rows;
            # the other qc's rows are garbage here but never read
            nc.vector.reciprocal_approx_fast(out=rg32, in_=sgrp)
            nc.vector.tensor_copy(out=rg, in_=rg32)
            _bcast_mul(AOb, rg, hgrp, qc)

        def attn_hqc(b, h, qc, AOb, sgrp, pump):
            """scores + exp + mask + AV + evac for one (batch, head, qc).
            pump(n) emits ~n x 0.4us of filler PE work."""
            thq = h // 2
            po = (h % 2) * 64
            even = (h % 2 == 0)
            loc = qc * 4 + (h % 4)
            q0 = b * S + qc * 512          # global m coords for QT/KT
            ql = qc * 512                  # batch-local q for AO
            nkt = (qc + 1) * 4
            ps_o = psO.tile([128, 512], f32, tag="psO")
            exts = [None] * nkt
            av_done = [0]

            def emit_avs(upto):
                while av_done[0] < upto:
                    kt = av_done[0]
                    ex, off = exts[kt]
                    nc.tensor.matmul(
                        ps_o[0:65, off:512],
                        V[:, b * 8 + kt, h * 65:h * 65 + 65],
                        ex[:, off:512],
                        start=(kt == 0), stop=(kt == nkt - 1))
                    av_done[0] += 1

            for pr in range(nkt // 2):
                ps_s = psS.tile([128, 1024], f32, tag="psS")
                for j in (0, 1):
                    kt = 2 * pr + j
                    k0 = kt * 128
                    off = max(0, k0 - qc * 512)
                    nc.tensor.matmul(
                        ps_s[:, j * 512 + off:j * 512 + 512],
                        KT[po:po + 64, thq, b * S + k0:b * S + k0 + 128],
                        QT[po:po + 64, thq, q0 + off:q0 + 512],
                        start=True, stop=True)
                for j in (0, 1):
                    kt = 2 * pr + j
                    k0 = kt * 128
                    off = max(0, k0 - qc * 512)
                    ex = expp.tile([128, 512], f16, tag="exp")
                    nc.scalar.activation(
                        out=ex[:, off:512],
                        in_=ps_s[:, j * 512 + off:j * 512 + 512],
                        func=EXPF, scale=0.125)
                    if k0 >= qc * 512:  # diagonal block: 0/1 triangle
                        nc.gpsimd.tensor_mul(
                            ex[:, off:off + 128],
                            ex[:, off:off + 128], tri01)
                    exts[kt] = (ex, off)
                if pr == 0:
                    pump(2)
                else:
                    emit_avs(2 * pr)
                    pump(1)
            emit_avs(nkt)

            # evacuate unnormalized output + sum row
            srow = srp.tile([65, 512], f32, tag="srow")
            nc.vector.tensor_copy(out=srow[64:65, :], in_=ps_o[64:65, :])
            nc.sync.dma_start(out=sgrp[loc:loc + 1, :], in_=srow[64:65, :])
            if even:
                nc.vector.tensor_copy(
                    out=AOb[0:64, thq, ql:ql + 512], in_=ps_o[0:64, :])
            else:
                tmp = tmpp.tile([64, 512], f16, tag="tmp")
                nc.vector.tensor_copy(out=tmp, in_=ps_o[0:64, :])
                nc.sync.dma_start(
                    out=AOb[64:128, thq, ql:ql + 512], in_=tmp)
            pump(1)

        # ---------------- emission schedule ----------------
        # phases A/B for batch 0, zip-interleaved so PE has work while the
        # input DMAs stream
        WvT = wt.tile([128, NDT, D], f16, tag="WT")
        drain(g_load_transposed(WvT, wv_d, D),
              g_load_transposed(xT0, x_d, S, row0=0))
        WqT = wt.tile([128, NDT, D], f16, tag="WT")
        drain2(g_proj_v(WvT, xT0, 0), g_load_transposed(WqT, wq_d, D))
        WkT = wt.tile([128, NDT, D], f16, tag="WT")
        drain2(g_proj_qk(WqT, xT0, QT, 0), g_load_transposed(WkT, wk_d, D))
        drain2(g_proj_qk(WkT, xT0, KT, 0),
               g_load_transposed(xT1, x_d, S, row0=S))

        # filler stream pumped between attention matmuls
        WoT = wt.tile([128, NDT, D], f16, tag="WT")  # ring slot of WqT
        fill = [chain(
            g_proj_qk(WqT, xT1, QT, 1),
            g_proj_qk(WkT, xT1, KT, 1),
            g_proj_v(WvT, xT1, 1),
            g_load_transposed(WoT, wo_d, D),
        )]

        def pump(n):
            for _ in range(n):
                if next(fill[0], _SENT) is _SENT:
                    return

        # attention batch 0 (QKV-b1 + WoT prep as filler)
        AO0 = xtao.tile([128, NDT, S], f16, tag="xTAO")  # ring slot of xT0
        for h in range(H):
            if h % 4 == 0:
                sgrp = grpp.tile([8, 512], f32, tag="sgrp")
            for qc in range(2):
                attn_hqc(0, h, qc, AO0, sgrp, pump)
            if h % 4 == 3:
                normalize_group(AO0, h // 4, sgrp)

        # drain remaining batch-1 projection work: it must complete before
        # AO1 (ring slot of xT1) can be written, or the psO ring deadlocks
        drain(fill[0])

        # attention batch 1 (out-proj b0 as filler); last head group runs
        # qc-major with split normalization so out-proj b1 starts early
        AO1 = xtao.tile([128, NDT, S], f16, tag="xTAO")  # ring slot of xT1
        fill[0] = g_outproj(AO0, WoT, 0, range(8))
        for h in range(12):
            if h % 4 == 0:
                sgrp = grpp.tile([8, 512], f32, tag="sgrp")
            for qc in range(2):
                attn_hqc(1, h, qc, AO1, sgrp, pump)
            if h % 4 == 3:
                normalize_group(AO1, h // 4, sgrp)
        sgrp = grpp.tile([8, 512], f32, tag="sgrp")
        for qc in range(2):
            for h in range(12, 16):
                attn_hqc(1, h, qc, AO1, sgrp, pump)
            normalize_qc(AO1, 3, sgrp, qc)

        # final out-proj b1: mts 0-3 depend only on the (long done) qc0
        # normalization, so they keep PE busy while the qc1 normalization
        # chain completes; mts 4-7 follow
        drain(fill[0], g_outproj(AO1, WoT, 1, range(8)))

    nc.compile()
    return nc


def _tri01():
    # tri01[dk, dq] = 1 where k <= q (allowed), else 0
    return np.triu(np.ones((128, 128), np.float16))


def _get_nc():
    if "nc" not in _CACHE:
        _CACHE["nc"] = _build_nc()
    return _CACHE["nc"]


def kernel(x, Wq, bq, Wk, bk, Wv, bv, Wo, bo):
    from concourse.bass_utils import run_bass_kernel_spmd

    x = np.ascontiguousarray(np.asarray(x, dtype=np.float32))
    B = x.shape[0]
    assert x.shape == (B, S, D) and B == NCORES * BPC
    Wq = np.ascontiguousarray(np.asarray(Wq, dtype=np.float32))
    Wk = np.ascontiguousarray(np.asarray(Wk, dtype=np.float32))
    Wv = np.ascontiguousarray(np.asarray(Wv, dtype=np.float32))
    Wo = np.ascontiguousarray(np.asarray(Wo, dtype=np.float32))

    nc = _get_nc()
    shards = x.reshape(NCORES, M, D)
    tri = _tri01()
    in_maps = [
        {"x": shards[c], "Wq": Wq, "Wk": Wk, "Wv": Wv, "Wo": Wo, "tri01": tri}
        for c in range(NCORES)
    ]
    res = run_bass_kernel_spmd(nc, in_maps, core_ids=list(range(NCORES)))
    y = np.stack([res.results[c]["y"] for c in range(NCORES)])
    y = y.reshape(B, S, D).astype(np.float32)

    # exact host-side fold of bv and bo (bq/bk are zero by problem spec)
    bias = (np.asarray(bv, np.float32) @ np.asarray(Wo, np.float32).T
            + np.asarray(bo, np.float32))
    if np.any(bias):
        y = y + bias
    return y.astype(np.float32)
